# revision 24
# baseline (speedup 1.0000x reference)
"""MoE batched-experts kernel for Trainium2 (8 NeuronCores, expert-parallel).

Problem: out[n] = sum_e routing[n,e] * MLP_e(x[n]) with N=4096, D=1024,
E=16 experts, H=2048, top-2 routing (routing_tensor is zero except for
each token's 2 chosen experts).

Strategy: only the chosen (token, expert) pairs are computed (8x fewer
FLOPs than the dense reference). Experts are sharded 2-per-core (largest
paired with smallest for load balance). The host gathers each expert's
tokens (transposed to [D, T], cast to bf16), the device runs the 2-layer
MLP with bf16 matmuls (full PE rate; fp32 PSUM accumulate) + exact-Gelu
on the scalar engine, scales columns by the routing score on the vector
engine, and the host scatter-adds the per-expert outputs back into the
[N, D] result.

Perf notes (from trace analysis):
- bf16 halves HBM traffic vs fp32r (16.7 MB weights/core) and avoids
  fp32r's unhidden 4-byte weight-load overhead; PE rate is identical.
- Both stages keep the matmul moving dim >=288: below ~110 columns the
  PE becomes LDWEIGHTS-limited, above it weight loads hide completely.
- Stage 2 computes y^T (d on partitions, tokens moving) so the PE cost
  is 128*T cycles exactly instead of rounding T up to a multiple of 128.
- Input tiles share SBUF buffers across the two expert slots (bufs=1
  pools): slot1's DMAs are issued right after the slot0 reads that free
  them, which staggers HBM traffic so the early critical loads (w0
  half0 + xt of slot0) get the full bandwidth.
- The sync and scalar HWDGE rings split the load; gpsimd (SWDGE) is
  unused. A few dummy matmuls over a zeroed tile at t=0 ride out the
  HAM clock-gate ramp while the first weights stream in.
"""

import math
import os

import numpy as np

P = 128
NE = 16  # experts
D = 1024
H = 2048

_CACHE: dict = {}
LAST_RESULTS = None  # BassKernelResults of the most recent device run


def _t_chunks(T):
    """Split T into equal-ish even moving-dim chunks <=512 (PSUM bank
    limit). Equal-ish keeps every chunk >=256, clear of the LDWEIGHTS-
    limited regime for small moving dims."""
    n = math.ceil(T / 512)
    base = T // n // 2 * 2
    out = []
    t = 0
    for i in range(n):
        c = base if i < n - 1 else T - base * (n - 1)
        out.append((t, c))
        t += c
    return out


def _build(T0: int, T1: int):
    """Build + compile the 2-expert-per-core MLP program.

    T0/T1: token count for expert slot 0 (large) / 1 (small).
    """
    import concourse.mybir as mybir
    import concourse.tile as tile
    from concourse import bacc

    F32 = mybir.dt.float32
    BF16 = mybir.dt.bfloat16
    AF = mybir.ActivationFunctionType
    MUL = mybir.AluOpType.mult

    KD = D // P   # 8 contraction chunks for x @ W0
    KH = H // P   # 16 contraction chunks for h @ W1
    HHALF = H // 2
    TS = T0 + T1

    nc = bacc.Bacc("TRN2", target_bir_lowering=False, debug=False, num_devices=8)
    # gathered tokens, transposed: col d*T+t = x[token t][d*128 + p]
    xt0 = nc.dram_tensor("xt0", [P, KD * T0], BF16, kind="ExternalInput").ap()
    xt1 = nc.dram_tensor("xt1", [P, KD * T1], BF16, kind="ExternalInput").ap()
    # W0 as [half, p, d*HHALF+j]; W1 as [p, h*D+j] (SBUF-layout-major so
    # each weight tile loads with a handful of large DMAs)
    w0a = nc.dram_tensor("w0a", [2, P, KD * HHALF], BF16, kind="ExternalInput").ap()
    w0b = nc.dram_tensor("w0b", [2, P, KD * HHALF], BF16, kind="ExternalInput").ap()
    w1a = nc.dram_tensor("w1a", [P, KH * D], BF16, kind="ExternalInput").ap()
    w1b = nc.dram_tensor("w1b", [P, KH * D], BF16, kind="ExternalInput").ap()
    # b0 per slot: [p, s*KH+h] = b0[h*128:(h+1)*128] of slot s
    b0s = nc.dram_tensor("b0s", [P, 2 * KH], F32, kind="ExternalInput").ap()
    # routing scores replicated across partitions: [p, t] (slot0 cols then
    # slot1 cols at offset T0)
    scl = nc.dram_tensor("scl", [P, TS], F32, kind="ExternalInput").ap()
    # output, transposed: y^T[d, t]
    y = nc.dram_tensor("y", [D, TS], BF16, kind="ExternalOutput").ap()

    with tile.TileContext(nc) as tc:
        with tc.tile_pool(name="wp", bufs=1) as wp, \
             tc.tile_pool(name="dp", bufs=1) as dp, \
             tc.tile_pool(name="op", bufs=6) as op, \
             tc.tile_pool(name="ps", bufs=8, space="PSUM") as ps:
            # PE warm-up: the HAM clock-gate runs the PE at reduced rate
            # for the first ~3us of activity; ride the ramp on dummy
            # matmuls while the first real DMAs are in flight.
            zt = dp.tile([P, 512], BF16, tag="zt")
            nc.vector.memset(zt[:], 0)
            pw = ps.tile([P, 512], F32, tag="ps", name="ps_warm")
            for i in range(7):
                nc.tensor.matmul(pw[:], zt[:, :P], zt[:], start=True, stop=True)

            for s, (xt_in, w0_in, w1_in, T) in enumerate(
                [(xt0, w0a, w1a, T0), (xt1, w0b, w1b, T1)]
            ):
                # --- this slot's tiles + input DMAs ---
                # Tags are shared across slots (bufs=1), so each slot-1 DMA
                # implicitly waits for the slot-0 read that frees its
                # buffer; issue order below keeps those waits off the
                # engine queues' critical path.
                xt = dp.tile([P, KD * T], BF16, tag="xt", name=f"xt{s}")
                w0t = [wp.tile([P, KD * HHALF], BF16, tag=f"w0_{h}",
                               name=f"w0_{s}_{h}") for h in range(2)]
                w1t = wp.tile([P, KH * D], BF16, tag="w1", name=f"w1_{s}")
                if s == 0:
                    # startup critical path: small leading pieces so the
                    # first matmuls wait on ~150-250 KB, not megabytes.
                    # sync ring: w0 (both halves); scalar ring: xt + consts
                    # + w1.
                    # piece sizes ramp with the d-loop's ~1.7us/d consumption:
                    # each ring issues a dma_start every ~2.2us and the
                    # completion semaphore fires ~2us after the last byte,
                    # so early pieces must be small and split across rings.
                    nc.sync.dma_start(w0t[0][:, :512], w0_in[0, :, :512])
                    nc.scalar.dma_start(xt[:, :T], xt_in[:, :T])
                    nc.sync.dma_start(w0t[0][:, 512:1536],
                                      w0_in[0, :, 512:1536])
                    nc.scalar.dma_start(xt[:, T:2 * T], xt_in[:, T:2 * T])
                    nc.sync.dma_start(w0t[0][:, 1536:3584],
                                      w0_in[0, :, 1536:3584])
                    nc.scalar.dma_start(xt[:, 2 * T:4 * T], xt_in[:, 2 * T:4 * T])
                    nc.sync.dma_start(w0t[0][:, 3584:],
                                      w0_in[0, :, 3584:])
                    nc.scalar.dma_start(xt[:, 4 * T:], xt_in[:, 4 * T:])
                    nc.sync.dma_start(w0t[1][:, :4 * HHALF],
                                      w0_in[1, :, :4 * HHALF])
                    nc.sync.dma_start(w0t[1][:, 4 * HHALF:],
                                      w0_in[1, :, 4 * HHALF:])
                    b0t = dp.tile([P, 2 * KH], F32, tag="b0t")
                    nc.scalar.dma_start(b0t[:], b0s)
                    sclt = dp.tile([P, TS], F32, tag="sclt")
                    nc.scalar.dma_start(sclt[:], scl)
                    for hq in range(2):
                        c0, c1 = 8 * hq * D, (8 * hq + 8) * D
                        nc.scalar.dma_start(w1t[:, c0:c1], w1_in[:, c0:c1])
                else:
                    # slot1 inputs were issued between slot0's stages (see
                    # below) — nothing to do here.
                    xt, w0t, w1t = xt_p, w0t_p, w1t_p

                hts = []
                for h in range(KH):
                    ht = dp.tile([P, T], BF16, tag=f"ht{h}", name=f"ht_s{s}_{h}")
                    hts.append(ht)

                # --- stage 1: hT[h, t] = gelu(x @ W0 + b0), h on partitions.
                for half in range(2):
                    for (t0, tw) in _t_chunks(T):
                        pt = []
                        for hh in range(KH // 2):
                            p_ = ps.tile([P, 512], F32, tag="ps",
                                         name=f"ps1_s{s}_{half}_{t0}_{hh}")
                            pt.append(p_)
                        for d in range(KD):
                            for hh in range(KH // 2):
                                nc.tensor.matmul(
                                    pt[hh][:, :tw],
                                    w0t[half][:, d * HHALF + hh * P:
                                              d * HHALF + (hh + 1) * P],
                                    xt[:, d * T + t0: d * T + t0 + tw],
                                    start=(d == 0), stop=(d == KD - 1),
                                )
                        for hh in range(KH // 2):
                            h = half * (KH // 2) + hh
                            nc.scalar.activation(
                                hts[h][:, t0:t0 + tw], pt[hh][:, :tw],
                                AF.Gelu, bias=b0t[:, s * KH + h: s * KH + h + 1],
                            )

                if s == 0:
                    # prefetch slot1 inputs: issued here (after slot0's
                    # stage-1 reads) so the tag-free waits are already
                    # satisfied or nearly so, and the transfers stream
                    # during slot0's stage 2.
                    T_n = T1
                    xt_p = dp.tile([P, KD * T_n], BF16, tag="xt", name="xt1")
                    w0t_p = [wp.tile([P, KD * HHALF], BF16, tag=f"w0_{h}",
                                     name=f"w0_1_{h}") for h in range(2)]
                    w1t_p = wp.tile([P, KH * D], BF16, tag="w1", name="w1_1")
                    nc.scalar.dma_start(xt_p[:, :4 * T_n], xt1[:, :4 * T_n])
                    nc.scalar.dma_start(xt_p[:, 4 * T_n:], xt1[:, 4 * T_n:])
                    for half in range(2):
                        nc.sync.dma_start(w0t_p[half][:, :4 * HHALF],
                                          w0b[half, :, :4 * HHALF])
                        nc.sync.dma_start(w0t_p[half][:, 4 * HHALF:],
                                          w0b[half, :, 4 * HHALF:])

                # --- stage 2: y^T[d, t] = s_t * (W1^T @ hT) ---
                ycol = 0 if s == 0 else T0
                for (t0, tw) in _t_chunks(T):
                    for dt in range(KD):
                        p2 = ps.tile([P, 512], F32, tag="ps",
                                     name=f"ps2_s{s}_{t0}_{dt}")
                        for h in range(KH):
                            nc.tensor.matmul(
                                p2[:, :tw],
                                w1t[:, h * D + dt * P: h * D + (dt + 1) * P],
                                hts[h][:, t0:t0 + tw],
                                start=(h == 0), stop=(h == KH - 1),
                            )
                        ot = op.tile([P, 512], BF16, tag="ot",
                                     name=f"ot_s{s}_{t0}_{dt}")
                        nc.vector.tensor_tensor(
                            ot[:, :tw], p2[:, :tw],
                            sclt[:, ycol + t0: ycol + t0 + tw], MUL)
                        nc.sync.dma_start(
                            y[dt * P:(dt + 1) * P, ycol + t0: ycol + t0 + tw],
                            ot[:, :tw])

                if s == 0:
                    # slot1's second-layer weights: tag frees at the end of
                    # slot0 stage 2 right above; streams during slot1
                    # stage 1.
                    for hq in range(2):
                        c0, c1 = 8 * hq * D, (8 * hq + 8) * D
                        nc.scalar.dma_start(w1t_p[:, c0:c1], w1b[:, c0:c1])

    nc.compile()
    return nc


def _ensure_ntff_hook_module():
    """bass_utils unconditionally imports antenv.axon_hooks when tracing is
    requested; on images without it, provide a shim wired to the axon
    ctypes profiler when available (else a no-hook fallback)."""
    import importlib.util
    import sys
    import types

    if importlib.util.find_spec("antenv") is None:
        return
    try:
        import antenv.axon_hooks  # noqa: F401
        return
    except ImportError:
        pass
    mod = types.ModuleType("antenv.axon_hooks")
    mod._hook = None

    def set_axon_ntff_profile_hook(h):
        mod._hook = h

    def get_axon_ntff_profile_hook():
        return mod._hook

    mod.set_axon_ntff_profile_hook = set_axon_ntff_profile_hook
    mod.get_axon_ntff_profile_hook = get_axon_ntff_profile_hook
    try:
        from trn_agent_boot.trn_boot import _ntff_profile_via_ctypes
        mod._hook = _ntff_profile_via_ctypes("/opt/axon/libaxon_pjrt.so")
    except Exception:
        pass
    sys.modules["antenv.axon_hooks"] = mod
    import antenv
    antenv.axon_hooks = mod


def kernel(x, routing_tensor, W0, b0, W1, b1):
    global LAST_RESULTS
    import ml_dtypes
    from concourse.bass_utils import run_bass_kernel_spmd
    _ensure_ntff_hook_module()

    BF16 = ml_dtypes.bfloat16
    x = np.ascontiguousarray(x, dtype=np.float32)
    routing = np.asarray(routing_tensor, dtype=np.float32)
    W0 = np.asarray(W0, dtype=np.float32)
    b0 = np.asarray(b0, dtype=np.float32)
    W1 = np.asarray(W1, dtype=np.float32)
    b1 = np.asarray(b1, dtype=np.float32)

    # --- routing prep: per-expert token lists ---
    idx = [np.nonzero(routing[:, e])[0] for e in range(NE)]
    counts = np.array([len(i) for i in idx])
    order = np.argsort(-counts, kind="stable")  # experts sorted by load desc
    big, small = order[:8], order[8:][::-1]     # pair rank i with rank 15-i
    # slot 0 (processed first) gets the SMALL expert: the startup-critical
    # phase then consumes weights at half the rate (512-col matmuls), which
    # the DMA rings can keep up with from a cold start.
    T0 = max(P, int(counts[small].max()) + 1 >> 1 << 1)
    T1 = max(P, int(counts[big].max()) + 1 >> 1 << 1)
    TS = T0 + T1

    key = (T0, T1)
    if key not in _CACHE:
        _CACHE[key] = _build(T0, T1)
    nc = _CACHE[key]

    # --- build per-core inputs ---
    in_maps = []
    KD, KH, HHALF = D // P, H // P, H // 2
    for c in range(8):
        ea, eb = int(small[c]), int(big[c])
        m = {}
        for name, e, T in (("xt0", ea, T0), ("xt1", eb, T1)):
            g = np.zeros((T, D), BF16)
            g[: len(idx[e])] = x[idx[e]].astype(BF16)
            # [P, d*T+t]: col d*T+t = x[token t][d*128+p]
            m[name] = np.ascontiguousarray(
                g.T.reshape(KD, P, T).transpose(1, 0, 2).reshape(P, KD * T))
        for name, e in (("w0a", ea), ("w0b", eb)):
            # [half, p, d*HHALF+j] = W0[e][d*128+p, half*HHALF+j]
            m[name] = np.ascontiguousarray(
                W0[e].astype(BF16).reshape(KD, P, 2, HHALF)
                .transpose(2, 1, 0, 3).reshape(2, P, KD * HHALF))
        for name, e in (("w1a", ea), ("w1b", eb)):
            # [p, h*D+j] = W1[e][h*128+p, j]
            m[name] = np.ascontiguousarray(
                W1[e].astype(BF16).reshape(KH, P, D)
                .transpose(1, 0, 2).reshape(P, KH * D))
        b0m = np.zeros((P, 2 * KH), np.float32)
        b0m[:, :KH] = b0[ea].reshape(KH, P).T
        b0m[:, KH:] = b0[eb].reshape(KH, P).T
        m["b0s"] = b0m
        sc = np.zeros((TS,), np.float32)
        sc[: len(idx[ea])] = routing[idx[ea], ea]
        sc[T0: T0 + len(idx[eb])] = routing[idx[eb], eb]
        m["scl"] = np.ascontiguousarray(
            np.broadcast_to(sc[None, :], (P, TS)))
        in_maps.append(m)

    res = run_bass_kernel_spmd(nc, in_maps, core_ids=list(range(8)),
                               trace=bool(os.environ.get("BASS_TRACE")))
    LAST_RESULTS = res

    # --- combine: out = routing @ b1 + scatter-add of per-expert rows ---
    out = routing @ b1
    for c in range(8):
        yc = res.results[c]["y"].astype(np.float32)  # [D, T0+T1]
        ea, eb = int(small[c]), int(big[c])
        out[idx[ea]] += yc[:, : len(idx[ea])].T
        out[idx[eb]] += yc[:, T0: T0 + len(idx[eb])].T
    return out.astype(np.float32)


# revision 28
# speedup vs baseline: 1.1721x; 1.1721x over previous
"""MoE batched-experts kernel for Trainium2 (8 NeuronCores, expert-parallel).

Problem: out[n] = sum_e routing[n,e] * MLP_e(x[n]) with N=4096, D=1024,
E=16 experts, H=2048, top-2 routing (routing_tensor is zero except for
each token's 2 chosen experts).

Strategy: only the chosen (token, expert) pairs are computed (8x fewer
FLOPs than the dense reference). Experts are sharded 2-per-core (largest
paired with smallest for load balance). The host gathers each expert's
tokens (transposed to [D, T], cast to bf16), the device runs the 2-layer
MLP with bf16 matmuls (full PE rate; fp32 PSUM accumulate) + exact-Gelu
on the scalar engine, scales columns by the routing score on the vector
engine, and the host scatter-adds the per-expert outputs back into the
[N, D] result.

Perf notes (from trace analysis):
- bf16 halves HBM traffic vs fp32r (16.7 MB weights/core) and avoids
  fp32r's unhidden 4-byte weight-load overhead; PE rate is identical.
- Both stages keep the matmul moving dim >=288: below ~110 columns the
  PE becomes LDWEIGHTS-limited, above it weight loads hide completely.
- Stage 2 computes y^T (d on partitions, tokens moving) so the PE cost
  is 128*T cycles exactly instead of rounding T up to a multiple of 128.
- Input tiles share SBUF buffers across the two expert slots (bufs=1
  pools): slot1's DMAs are issued right after the slot0 reads that free
  them, which staggers HBM traffic so the early critical loads (w0
  half0 + xt of slot0) get the full bandwidth.
- The sync and scalar HWDGE rings split the load; gpsimd (SWDGE) is
  unused. A few dummy matmuls over a zeroed tile at t=0 ride out the
  HAM clock-gate ramp while the first weights stream in.
"""

import math
import os

import numpy as np

P = 128
NE = 16  # experts
D = 1024
H = 2048

_CACHE: dict = {}
LAST_RESULTS = None  # BassKernelResults of the most recent device run


def _t_chunks(T):
    """Split T into equal-ish even moving-dim chunks <=512 (PSUM bank
    limit). Equal-ish keeps every chunk >=256, clear of the LDWEIGHTS-
    limited regime for small moving dims."""
    n = math.ceil(T / 512)
    base = T // n // 2 * 2
    out = []
    t = 0
    for i in range(n):
        c = base if i < n - 1 else T - base * (n - 1)
        out.append((t, c))
        t += c
    return out


def _build(T0: int, T1: int):
    """Build + compile the 2-expert-per-core MLP program.

    T0/T1: token count for expert slot 0 (large) / 1 (small).
    """
    import concourse.mybir as mybir
    import concourse.tile as tile
    from concourse import bacc

    F32 = mybir.dt.float32
    BF16 = mybir.dt.bfloat16
    AF = mybir.ActivationFunctionType
    MUL = mybir.AluOpType.mult

    KD = D // P   # 8 contraction chunks for x @ W0
    KH = H // P   # 16 contraction chunks for h @ W1
    HHALF = H // 2
    TS = T0 + T1

    nc = bacc.Bacc("TRN2", target_bir_lowering=False, debug=False, num_devices=8)
    # gathered tokens, transposed: col d*T+t = x[token t][d*128 + p]
    xt0 = nc.dram_tensor("xt0", [P, KD * T0], BF16, kind="ExternalInput").ap()
    xt1 = nc.dram_tensor("xt1", [P, KD * T1], BF16, kind="ExternalInput").ap()
    # W0 as [half, p, d*HHALF+j]; W1 as [p, h*D+j] (SBUF-layout-major so
    # each weight tile loads with a handful of large DMAs)
    w0a = nc.dram_tensor("w0a", [2, P, KD * HHALF], BF16, kind="ExternalInput").ap()
    w0b = nc.dram_tensor("w0b", [2, P, KD * HHALF], BF16, kind="ExternalInput").ap()
    w1a = nc.dram_tensor("w1a", [P, KH * D], BF16, kind="ExternalInput").ap()
    w1b = nc.dram_tensor("w1b", [P, KH * D], BF16, kind="ExternalInput").ap()
    # b0 per slot: [p, s*KH+h] = b0[h*128:(h+1)*128] of slot s
    b0s = nc.dram_tensor("b0s", [P, 2 * KH], F32, kind="ExternalInput").ap()
    # routing scores replicated across partitions: [p, t] (slot0 cols then
    # slot1 cols at offset T0)
    scl = nc.dram_tensor("scl", [P, TS], F32, kind="ExternalInput").ap()
    # output, transposed: y^T[d, t]
    y = nc.dram_tensor("y", [D, TS], BF16, kind="ExternalOutput").ap()

    with tile.TileContext(nc) as tc:
        with tc.tile_pool(name="wp", bufs=1) as wp, \
             tc.tile_pool(name="dp", bufs=1) as dp, \
             tc.tile_pool(name="op", bufs=6) as op, \
             tc.tile_pool(name="ps", bufs=8, space="PSUM") as ps:
            # PE warm-up: the HAM clock-gate runs the PE at reduced rate
            # for the first ~3us of activity; ride the ramp on dummy
            # matmuls while the first real DMAs are in flight.
            zt = dp.tile([P, 512], BF16, tag="zt")
            nc.vector.memset(zt[:], 0)
            pw = ps.tile([P, 512], F32, tag="ps", name="ps_warm")
            for i in range(7):
                nc.tensor.matmul(pw[:], zt[:, :P], zt[:], start=True, stop=True)

            for s, (xt_in, w0_in, w1_in, T) in enumerate(
                [(xt0, w0a, w1a, T0), (xt1, w0b, w1b, T1)]
            ):
                # --- this slot's tiles + input DMAs ---
                # Tags are shared across slots (bufs=1), so each slot-1 DMA
                # implicitly waits for the slot-0 read that frees its
                # buffer; issue order below keeps those waits off the
                # engine queues' critical path.
                xt = dp.tile([P, KD * T], BF16, tag="xt", name=f"xt{s}")
                w0t = [wp.tile([P, KD * HHALF], BF16, tag=f"w0_{h}",
                               name=f"w0_{s}_{h}") for h in range(2)]
                w1t = wp.tile([P, KH * D], BF16, tag="w1", name=f"w1_{s}")
                if s == 0:
                    # startup critical path: small leading pieces so the
                    # first matmuls wait on ~150-250 KB, not megabytes.
                    # sync ring: w0 (both halves); scalar ring: xt + consts
                    # + w1.
                    # piece sizes ramp with the d-loop's ~1.7us/d consumption:
                    # each ring issues a dma_start every ~2.2us and the
                    # completion semaphore fires ~2us after the last byte,
                    # so early pieces must be small and split across rings.
                    nc.sync.dma_start(w0t[0][:, :512], w0_in[0, :, :512])
                    nc.scalar.dma_start(xt[:, :T], xt_in[:, :T])
                    nc.sync.dma_start(w0t[0][:, 512:1536],
                                      w0_in[0, :, 512:1536])
                    nc.scalar.dma_start(xt[:, T:2 * T], xt_in[:, T:2 * T])
                    nc.sync.dma_start(w0t[0][:, 1536:3584],
                                      w0_in[0, :, 1536:3584])
                    nc.scalar.dma_start(xt[:, 2 * T:4 * T], xt_in[:, 2 * T:4 * T])
                    nc.sync.dma_start(w0t[0][:, 3584:],
                                      w0_in[0, :, 3584:])
                    nc.scalar.dma_start(xt[:, 4 * T:], xt_in[:, 4 * T:])
                    nc.sync.dma_start(w0t[1][:, :4 * HHALF],
                                      w0_in[1, :, :4 * HHALF])
                    nc.sync.dma_start(w0t[1][:, 4 * HHALF:],
                                      w0_in[1, :, 4 * HHALF:])
                    b0t = dp.tile([P, 2 * KH], F32, tag="b0t")
                    nc.scalar.dma_start(b0t[:], b0s)
                    sclt = dp.tile([P, TS], F32, tag="sclt")
                    nc.scalar.dma_start(sclt[:], scl)
                    for hq in range(2):
                        c0, c1 = 8 * hq * D, (8 * hq + 8) * D
                        nc.scalar.dma_start(w1t[:, c0:c1], w1_in[:, c0:c1])
                else:
                    # slot1 inputs were issued between slot0's stages (see
                    # below) — nothing to do here.
                    xt, w0t, w1t = xt_p, w0t_p, w1t_p

                hts = []
                for h in range(KH):
                    ht = dp.tile([P, T], BF16, tag=f"ht{h}", name=f"ht_s{s}_{h}")
                    hts.append(ht)

                # --- stage 1: hT[h, t] = gelu(x @ W0 + b0), h on partitions.
                for half in range(2):
                    for (t0, tw) in _t_chunks(T):
                        pt = []
                        for hh in range(KH // 2):
                            p_ = ps.tile([P, 512], F32, tag="ps",
                                         name=f"ps1_s{s}_{half}_{t0}_{hh}")
                            pt.append(p_)
                        for d in range(KD):
                            for hh in range(KH // 2):
                                nc.tensor.matmul(
                                    pt[hh][:, :tw],
                                    w0t[half][:, d * HHALF + hh * P:
                                              d * HHALF + (hh + 1) * P],
                                    xt[:, d * T + t0: d * T + t0 + tw],
                                    start=(d == 0), stop=(d == KD - 1),
                                )
                        for hh in range(KH // 2):
                            h = half * (KH // 2) + hh
                            nc.scalar.activation(
                                hts[h][:, t0:t0 + tw], pt[hh][:, :tw],
                                AF.Gelu, bias=b0t[:, s * KH + h: s * KH + h + 1],
                            )

                if s == 0:
                    # prefetch slot1 inputs: issued here (after slot0's
                    # stage-1 reads) so the tag-free waits are already
                    # satisfied or nearly so, and the transfers stream
                    # during slot0's stage 2.
                    T_n = T1
                    xt_p = dp.tile([P, KD * T_n], BF16, tag="xt", name="xt1")
                    w0t_p = [wp.tile([P, KD * HHALF], BF16, tag=f"w0_{h}",
                                     name=f"w0_1_{h}") for h in range(2)]
                    w1t_p = wp.tile([P, KH * D], BF16, tag="w1", name="w1_1")
                    nc.scalar.dma_start(xt_p[:, :4 * T_n], xt1[:, :4 * T_n])
                    nc.scalar.dma_start(xt_p[:, 4 * T_n:], xt1[:, 4 * T_n:])
                    for half in range(2):
                        nc.sync.dma_start(w0t_p[half][:, :4 * HHALF],
                                          w0b[half, :, :4 * HHALF])
                        nc.sync.dma_start(w0t_p[half][:, 4 * HHALF:],
                                          w0b[half, :, 4 * HHALF:])

                # --- stage 2: y^T[d, t] = s_t * (W1^T @ hT) ---
                ycol = 0 if s == 0 else T0
                for (t0, tw) in _t_chunks(T):
                    for dt in range(KD):
                        p2 = ps.tile([P, 512], F32, tag="ps",
                                     name=f"ps2_s{s}_{t0}_{dt}")
                        for h in range(KH):
                            nc.tensor.matmul(
                                p2[:, :tw],
                                w1t[:, h * D + dt * P: h * D + (dt + 1) * P],
                                hts[h][:, t0:t0 + tw],
                                start=(h == 0), stop=(h == KH - 1),
                            )
                        ot = op.tile([P, 512], BF16, tag="ot",
                                     name=f"ot_s{s}_{t0}_{dt}")
                        nc.vector.tensor_tensor(
                            ot[:, :tw], p2[:, :tw],
                            sclt[:, ycol + t0: ycol + t0 + tw], MUL)
                        nc.sync.dma_start(
                            y[dt * P:(dt + 1) * P, ycol + t0: ycol + t0 + tw],
                            ot[:, :tw])

                if s == 0:
                    # slot1's second-layer weights: tag frees at the end of
                    # slot0 stage 2 right above; streams during slot1
                    # stage 1.
                    for hq in range(2):
                        c0, c1 = 8 * hq * D, (8 * hq + 8) * D
                        nc.scalar.dma_start(w1t_p[:, c0:c1], w1b[:, c0:c1])

    nc.compile()
    return nc


def _ensure_ntff_hook_module():
    """bass_utils unconditionally imports antenv.axon_hooks when tracing is
    requested; on images without it, provide a shim wired to the axon
    ctypes profiler when available (else a no-hook fallback)."""
    import importlib.util
    import sys
    import types

    if importlib.util.find_spec("antenv") is None:
        return
    try:
        import antenv.axon_hooks  # noqa: F401
        return
    except ImportError:
        pass
    mod = types.ModuleType("antenv.axon_hooks")
    mod._hook = None

    def set_axon_ntff_profile_hook(h):
        mod._hook = h

    def get_axon_ntff_profile_hook():
        return mod._hook

    mod.set_axon_ntff_profile_hook = set_axon_ntff_profile_hook
    mod.get_axon_ntff_profile_hook = get_axon_ntff_profile_hook
    try:
        from trn_agent_boot.trn_boot import _ntff_profile_via_ctypes
        mod._hook = _ntff_profile_via_ctypes("/opt/axon/libaxon_pjrt.so")
    except Exception:
        pass
    sys.modules["antenv.axon_hooks"] = mod
    import antenv
    antenv.axon_hooks = mod


def kernel(x, routing_tensor, W0, b0, W1, b1):
    global LAST_RESULTS
    import ml_dtypes
    from concourse.bass_utils import run_bass_kernel_spmd
    _ensure_ntff_hook_module()

    BF16 = ml_dtypes.bfloat16
    x = np.ascontiguousarray(x, dtype=np.float32)
    routing = np.asarray(routing_tensor, dtype=np.float32)
    W0 = np.asarray(W0, dtype=np.float32)
    b0 = np.asarray(b0, dtype=np.float32)
    W1 = np.asarray(W1, dtype=np.float32)
    b1 = np.asarray(b1, dtype=np.float32)

    # --- routing prep: per-expert token lists ---
    idx = [np.nonzero(routing[:, e])[0] for e in range(NE)]
    counts = np.array([len(i) for i in idx])
    order = np.argsort(-counts, kind="stable")  # experts sorted by load desc
    big, small = order[:8], order[8:][::-1]     # pair rank i with rank 15-i
    # slot 0 (processed first) gets the SMALL expert: the startup-critical
    # phase then consumes weights at half the rate (512-col matmuls), which
    # the DMA rings can keep up with from a cold start.
    T0 = max(P, int(counts[small].max()) + 1 >> 1 << 1)
    T1 = max(P, int(counts[big].max()) + 1 >> 1 << 1)
    TS = T0 + T1

    key = (T0, T1)
    if key not in _CACHE:
        _CACHE[key] = _build(T0, T1)
    nc = _CACHE[key]

    # --- build per-core inputs ---
    in_maps = []
    KD, KH, HHALF = D // P, H // P, H // 2
    for c in range(8):
        ea, eb = int(small[c]), int(big[c])
        m = {}
        for name, e, T in (("xt0", ea, T0), ("xt1", eb, T1)):
            g = np.zeros((T, D), BF16)
            g[: len(idx[e])] = x[idx[e]].astype(BF16)
            # [P, d*T+t]: col d*T+t = x[token t][d*128+p]
            m[name] = np.ascontiguousarray(
                g.T.reshape(KD, P, T).transpose(1, 0, 2).reshape(P, KD * T))
        for name, e in (("w0a", ea), ("w0b", eb)):
            # [half, p, d*HHALF+j] = W0[e][d*128+p, half*HHALF+j]
            m[name] = np.ascontiguousarray(
                W0[e].astype(BF16).reshape(KD, P, 2, HHALF)
                .transpose(2, 1, 0, 3).reshape(2, P, KD * HHALF))
        for name, e in (("w1a", ea), ("w1b", eb)):
            # [p, h*D+j] = W1[e][h*128+p, j]
            m[name] = np.ascontiguousarray(
                W1[e].astype(BF16).reshape(KH, P, D)
                .transpose(1, 0, 2).reshape(P, KH * D))
        b0m = np.zeros((P, 2 * KH), np.float32)
        b0m[:, :KH] = b0[ea].reshape(KH, P).T
        b0m[:, KH:] = b0[eb].reshape(KH, P).T
        m["b0s"] = b0m
        sc = np.zeros((TS,), np.float32)
        sc[: len(idx[ea])] = routing[idx[ea], ea]
        sc[T0: T0 + len(idx[eb])] = routing[idx[eb], eb]
        m["scl"] = np.ascontiguousarray(
            np.broadcast_to(sc[None, :], (P, TS)))
        in_maps.append(m)

    res = run_bass_kernel_spmd(nc, in_maps, core_ids=list(range(8)),
                               trace=bool(os.environ.get("BASS_TRACE")))
    LAST_RESULTS = res

    # --- combine: out = routing @ b1 + scatter-add of per-expert rows ---
    out = routing @ b1
    for c in range(8):
        yc = res.results[c]["y"].astype(np.float32)  # [D, T0+T1]
        ea, eb = int(small[c]), int(big[c])
        out[idx[ea]] += yc[:, : len(idx[ea])].T
        out[idx[eb]] += yc[:, T0: T0 + len(idx[eb])].T
    return out.astype(np.float32)
